# revision 8
# baseline (speedup 1.0000x reference)
"""Trainium2 Bass kernel for nn_ComplexRecurrentSequenceNetwork.

Self-contained: builds, compiles and runs the Bass kernel on 8 NeuronCores,
data-parallel over the batch dim (1 batch element per core, weights replicated).

Algebraic collapse used (verified vs the jax reference to fp32 noise level):
  - All K=16 stack slots stay identical (update is slot-uniform from zero init),
    so the memory attention reduces to the Mv complex-linear of a single [N,2D]
    state; Mq/Mk are dead weights.
  - The pointer only enters through its sum s, with s_{t+1} = s_t*gs/(gs+eps);
    |1-s| <= ~1e-5 over 8 steps, so s==1 (error far below fp32 noise).
  - read = 0.1*Mv(m)*s tracked directly as state V: V' = (1-p)V + p*w,
    w = 0.1*Mv(zf); z' = zf + V'.

Numerics: main path (q/k/v projections, scores, exp, AV, rowsum) in fp32
(the recurrence is chaotic: ~2.5x/step error growth; fp32r there gives ~3%
final error, fp32 gives ~7e-4).  Gate/w/broadcast matmuls in fp32r (verified
insensitive).  Softmax max-pass in bf16 (shift only needs +-40 accuracy),
with the row-max folded into the transposed-scores matmul as an appended
rank-1 contraction term.
"""
import sys

try:
    import concourse.bass as bass  # noqa: F401
except ImportError:
    sys.path.insert(0, "/opt/trn_rl_repo")

import numpy as np

import concourse.bacc as bacc
import concourse.bass as bass
import concourse.mybir as mybir
import concourse.tile as tile
from concourse.alu_op_type import AluOpType
from concourse.bass_utils import run_bass_kernel_spmd

F32 = mybir.dt.float32
F32R = mybir.dt.float32r
BF16 = mybir.dt.bfloat16
AF = mybir.ActivationFunctionType
AX = mybir.AxisListType

S, D, TWO_D, NB = 1024, 128, 256, 2   # seq len, dim, 2*dim, 512-col blocks
BLK = S // NB                          # 512
JC = S // 128                          # 8 token chunks of 128
SCALE = float(D) ** -0.5
EPS = 1e-6


def _r(ap):
    return ap.bitcast(F32R)


def build_nc(T=8):
    nc = bacc.Bacc("TRN2", target_bir_lowering=False, debug=False)

    dram = {}
    def din(name, shape):
        dram[name] = nc.dram_tensor(name, shape, F32, kind="ExternalInput")
        return dram[name]

    xr_d = din("xr", [S, D]); xi_d = din("xi", [S, D])
    # q/k stationary weights ([in, out]); *_n = negated imag part
    for n in ["wqr", "wqi", "wqin", "wkr", "wki", "wkin"]:
        din(n, [D, D])
    din("wv1", [D, TWO_D]); din("wv2", [D, TWO_D])       # v moving weights
    for n in ["mr", "mi", "min"]:                         # 0.1*Mv variants
        din(n, [D, D])
    din("wc1", [D, 3]); din("wc2", [D, 3]); din("bch", [3, 1])
    din("wo1", [D, D]); din("wo2", [D, D]); din("bo", [D, 1])
    din("ident", [D, D])
    din("ones_c", [128, 1])                               # rowsum lhsT (fp32)
    din("ones_r", [1, 128])                               # bcast/append lhsT
    din("ones3", [3, 1])
    out_d = nc.dram_tensor("out", [S, D], F32, kind="ExternalOutput")

    with tile.TileContext(nc) as tc:
        _emit(nc, tc, dram, out_d, T)
    nc.compile()
    return nc


def _t(pool, shape, dtype, tag):
    return pool.tile(shape, dtype, tag=tag, name=tag)


def _emit(nc, tc, dram, out_d, T):
    import contextlib
    ctx = contextlib.ExitStack()
    with ctx:
        cp = ctx.enter_context(tc.tile_pool(name="consts", bufs=1))
        st = ctx.enter_context(tc.tile_pool(name="state", bufs=1))
        wk = ctx.enter_context(tc.tile_pool(name="work", bufs=1))
        rowp = ctx.enter_context(tc.tile_pool(name="rows", bufs=2))
        pmm = ctx.enter_context(tc.tile_pool(name="pmm", bufs=2, space="PSUM"))
        pacc = ctx.enter_context(tc.tile_pool(name="pacc", bufs=4, space="PSUM"))
        paux = ctx.enter_context(tc.tile_pool(name="paux", bufs=2, space="PSUM"))

        # ---- load constants ----
        c = {}
        for n, sh in [("wqr", [D, D]), ("wqi", [D, D]), ("wqin", [D, D]),
                      ("wkr", [D, D]), ("wki", [D, D]), ("wkin", [D, D]),
                      ("wv1", [D, TWO_D]), ("wv2", [D, TWO_D]),
                      ("wo1", [D, D]), ("wo2", [D, D]),
                      ("ident", [D, D]), ("ones_c", [128, 1]),
                      ("ones_r", [1, 128]),
                      ("bch", [3, 1]), ("bo", [D, 1])]:
            c[n] = _t(cp, sh, F32, n)
            nc.sync.dma_start(out=c[n], in_=dram[n][:, :])
        # f32r-rounded constants (producers must round for fp32r matmuls)
        for n, sh in [("mr", [D, D]), ("mi", [D, D]), ("min", [D, D]),
                      ("wc1", [D, 3]), ("wc2", [D, 3]), ("ones3", [3, 1])]:
            raw = _t(cp, sh, F32, n + "_raw")
            nc.sync.dma_start(out=raw, in_=dram[n][:, :])
            c[n] = _t(cp, sh, F32, n)
            nc.vector.tensor_copy(out=_r(c[n]), in_=raw)
        c["ones_rb"] = _t(cp, [1, 128], BF16, "ones_rb")
        nc.vector.tensor_copy(out=c["ones_rb"], in_=c["ones_r"])

        # ---- state (parity buffered) ----
        z = [[_t(st, [128, S], F32, f"z{p}{comp}") for comp in "ri"] for p in (0, 1)]
        V = [[_t(st, [128, S], F32, f"V{p}{comp}") for comp in "ri"] for p in (0, 1)]
        for comp in (0, 1):
            nc.vector.memset(V[0][comp], 0.0)

        # ---- load + transpose x into z[0] ([feat, tok] layout) ----
        for comp, src in [(0, dram["xr"]), (1, dram["xi"])]:
            x_sb = _t(wk, [128, JC, D], F32, "xin")
            nc.sync.dma_start(out=x_sb, in_=src.rearrange("(c p) d -> p c d", p=128))
            for ch in range(JC):
                ps = _t(pmm, [128, 128], F32, "mm")
                nc.tensor.transpose(ps, x_sb[:, ch, :], c["ident"])
                nc.scalar.copy(out=z[0][comp][:, ch * 128:(ch + 1) * 128], in_=ps)

        # ---- work tiles ----
        qk = {n: _t(wk, [128, S], F32, n) for n in ["qrT", "qiT", "krT", "kiT"]}
        qk_bf = {n: _t(wk, [128, S], BF16, n + "bf") for n in ["qrT", "qiT", "krT", "kiT"]}
        v_sb = _t(wk, [128, JC, TWO_D], F32, "vsb")
        e_sb = _t(wk, [128, JC, S], F32, "esb")
        mcol = _t(wk, [128, JC], F32, "mcol")
        zf = [_t(wk, [128, S], F32, f"zf{comp}") for comp in "ri"]
        zfr_ = [_t(wk, [128, S], F32, f"zfr{comp}") for comp in "ri"]  # f32r copies
        tg = _t(wk, [3, S], F32, "tg")
        negm = _t(wk, [1, S], BF16, "negm")
        dtl = [_t(wk, [128, S], F32, f"d{comp}") for comp in "ri"]
        t2l = [_t(wk, [128, S], F32, f"t2{comp}") for comp in "ri"]
        rcpb = _t(wk, [128, S], F32, "rcpb")

        for t in range(T):
            za, Va = z[t % 2], V[t % 2]
            zb, Vb = z[(t + 1) % 2], V[(t + 1) % 2]

            # --- phase A: q/k projections (fp32), [feat, tok] layout ---
            specs = [("qrT", "wqr", 0, "wqin", 1), ("qiT", "wqi", 0, "wqr", 1),
                     ("krT", "wkr", 0, "wkin", 1), ("kiT", "wki", 0, "wkr", 1)]
            for b in range(NB):
                sl = slice(b * BLK, (b + 1) * BLK)
                for dst, wA, sA, wB, sB in specs:
                    ps = _t(pmm, [128, BLK], F32, "mm")
                    nc.tensor.matmul(ps, c[wA], za[sA][:, sl], start=True, stop=False)
                    nc.tensor.matmul(ps, c[wB], za[sB][:, sl], start=False, stop=True)
                    nc.scalar.copy(out=qk[dst][:, sl], in_=ps)
            # bf16 copies for the max-pass (gpsimd: SBUF-only engine, off DVE/ACT)
            for n in ["qrT", "qiT", "krT", "kiT"]:
                nc.gpsimd.tensor_copy(out=qk_bf[n], in_=qk[n])

            # --- phase B: v natural layout [tok, 2D] (fp32) ---
            for ch in range(JC):
                csl = slice(ch * 128, (ch + 1) * 128)
                ps = _t(pmm, [128, TWO_D], F32, "mm")
                nc.tensor.matmul(ps, za[0][:, csl], c["wv1"], start=True, stop=False)
                nc.tensor.matmul(ps, za[1][:, csl], c["wv2"], start=False, stop=True)
                nc.scalar.copy(out=v_sb[:, ch, :], in_=ps)

            # --- phase C: bf16 max-pass, scores in [i, j] layout ---
            for ic in range(JC):
                isl = slice(ic * 128, (ic + 1) * 128)
                r01 = _t(rowp, [128, 2], F32, "mr2")
                for jb in range(NB):
                    jsl = slice(jb * BLK, (jb + 1) * BLK)
                    ps = _t(pmm, [128, BLK], F32, "mm")
                    nc.tensor.matmul(ps, qk_bf["qrT"][:, isl], qk_bf["krT"][:, jsl],
                                     start=True, stop=False)
                    nc.tensor.matmul(ps, qk_bf["qiT"][:, isl], qk_bf["kiT"][:, jsl],
                                     start=False, stop=True)
                    nc.vector.tensor_reduce(out=r01[:, jb:jb + 1], in_=ps,
                                            axis=AX.X, op=AluOpType.max)
                nc.vector.tensor_tensor(out=mcol[:, ic:ic + 1], in0=r01[:, 0:1],
                                        in1=r01[:, 1:2], op=AluOpType.max)
            # row-max columns -> one row [1, S] (PE transpose), negated f32r copy
            for b in range(NB):
                ps_row = _t(paux, [1, BLK], F32, "aux")
                for k in range(4):
                    ic = b * 4 + k
                    nc.tensor.transpose(ps_row[0:1, k * 128:(k + 1) * 128],
                                        mcol[:, ic:ic + 1], c["ident"])
                nc.scalar.activation(out=negm[0:1, b * BLK:(b + 1) * BLK],
                                     in_=ps_row, func=AF.Copy, scale=-1.0)

            # --- phase D: transposed scores + shift + exp (fp32 + f32r shift) ---
            for ch in range(JC):
                csl = slice(ch * 128, (ch + 1) * 128)
                for b in range(NB):
                    sl = slice(b * BLK, (b + 1) * BLK)
                    ps = _t(pmm, [128, BLK], F32, "mm")
                    nc.tensor.matmul(ps, qk["krT"][:, csl], qk["qrT"][:, sl],
                                     start=True, stop=False)
                    nc.tensor.matmul(ps, qk["kiT"][:, csl], qk["qiT"][:, sl],
                                     start=False, stop=False)
                    nc.tensor.matmul(ps, c["ones_rb"], negm[0:1, sl],
                                     start=False, stop=True)
                    nc.scalar.activation(out=e_sb[:, ch, sl], in_=ps,
                                         func=AF.Exp, scale=SCALE)

            # --- phase E: AV + rowsum (fp32), accumulate over j chunks ---
            ar_ps, rs_ps = [], []
            for b in range(NB):
                sl = slice(b * BLK, (b + 1) * BLK)
                pr = _t(pacc, [128, BLK], F32, "acc")
                pi = _t(pacc, [128, BLK], F32, "acc")
                rs = _t(paux, [1, BLK], F32, "aux")
                for ch in range(JC):
                    e = e_sb[:, ch, sl]
                    nc.tensor.matmul(pr, v_sb[:, ch, 0:D], e,
                                     start=(ch == 0), stop=(ch == JC - 1))
                    nc.tensor.matmul(pi, v_sb[:, ch, D:TWO_D], e,
                                     start=(ch == 0), stop=(ch == JC - 1))
                    nc.tensor.matmul(rs, c["ones_c"], e,
                                     start=(ch == 0), stop=(ch == JC - 1))
                ar_ps.append((pr, pi)); rs_ps.append(rs)

            # --- phase F: normalize -> zf (+ f32r copies for gate/w matmuls) ---
            for b in range(NB):
                sl = slice(b * BLK, (b + 1) * BLK)
                rcp = _t(rowp, [1, BLK], F32, "rcp")
                nc.vector.reciprocal_approx_fast(out=rcp, in_=rs_ps[b])
                psb = _t(paux, [128, BLK], F32, "aux")
                nc.tensor.matmul(psb, c["ones_r"], rcp, start=True, stop=True)
                nc.scalar.copy(out=rcpb[:, sl], in_=psb)
                for comp in (0, 1):
                    nc.vector.tensor_tensor(out=zf[comp][:, sl], in0=ar_ps[b][comp],
                                            in1=rcpb[:, sl], op=AluOpType.mult)
                    nc.vector.tensor_copy(out=_r(zfr_[comp][:, sl]), in_=zf[comp][:, sl])

            # --- phase G: gates (f32r) ---
            for b in range(NB):
                sl = slice(b * BLK, (b + 1) * BLK)
                gp = _t(pmm, [3, BLK], F32, "mm")
                nc.tensor.matmul(gp, _r(c["wc1"]), _r(zfr_[0][:, sl]), start=True, stop=False)
                nc.tensor.matmul(gp, _r(c["wc2"]), _r(zfr_[1][:, sl]), start=False, stop=True)
                nc.scalar.activation(out=_r(tg[:, sl]), in_=gp, func=AF.Tanh,
                                     scale=0.5, bias=c["bch"])
                tp = _t(paux, [1, BLK], F32, "aux")
                nc.tensor.matmul(tp, _r(c["ones3"]), _r(tg[:, sl]), start=True, stop=True)
                drow = _t(rowp, [1, BLK], F32, "drow")
                nc.vector.tensor_scalar_add(out=drow, in0=tp, scalar1=float(3.0 + 2 * EPS))
                rd = _t(rowp, [1, BLK], F32, "rd")
                nc.vector.reciprocal_approx_fast(out=rd, in_=drow)
                prow = _t(rowp, [1, BLK], F32, "prow")
                nc.vector.scalar_tensor_tensor(out=prow, in0=tg[0:1, sl], scalar=1.0,
                                               in1=rd, op0=AluOpType.add, op1=AluOpType.mult)
                pb = _t(paux, [128, BLK], F32, "aux")
                nc.tensor.matmul(pb, c["ones_r"], prow, start=True, stop=True)

                # --- phase H: w (f32r) + V update + z' ---
                wps = []
                for wA, wB in [("mr", "min"), ("mi", "mr")]:
                    wp = _t(pmm, [128, BLK], F32, "mm")
                    nc.tensor.matmul(wp, _r(c[wA]), _r(zfr_[0][:, sl]), start=True, stop=False)
                    nc.tensor.matmul(wp, _r(c[wB]), _r(zfr_[1][:, sl]), start=False, stop=True)
                    wps.append(wp)
                for comp in (0, 1):
                    nc.vector.tensor_tensor(out=dtl[comp][:, sl], in0=wps[comp],
                                            in1=Va[comp][:, sl], op=AluOpType.subtract)
                    nc.vector.tensor_tensor(out=t2l[comp][:, sl], in0=dtl[comp][:, sl],
                                            in1=pb, op=AluOpType.mult)
                    nc.gpsimd.tensor_tensor(out=Vb[comp][:, sl], in0=Va[comp][:, sl],
                                            in1=t2l[comp][:, sl], op=AluOpType.add)
                    nc.vector.tensor_tensor(out=zb[comp][:, sl], in0=zf[comp][:, sl],
                                            in1=Vb[comp][:, sl], op=AluOpType.add)

        # ---- epilogue: out = [zr|zi] @ Wo + bo, back to [tok, feat] ----
        zfin = z[T % 2]
        oT = _t(wk, [128, S], F32, "oT")
        for b in range(NB):
            sl = slice(b * BLK, (b + 1) * BLK)
            ps = _t(pmm, [128, BLK], F32, "mm")
            nc.tensor.matmul(ps, c["wo1"], zfin[0][:, sl], start=True, stop=False)
            nc.tensor.matmul(ps, c["wo2"], zfin[1][:, sl], start=False, stop=True)
            nc.scalar.activation(out=oT[:, sl], in_=ps, func=AF.Identity, bias=c["bo"])
        o_sb = _t(wk, [128, JC, D], F32, "onat")
        for ch in range(JC):
            ps = _t(pmm, [128, 128], F32, "mm")
            nc.tensor.transpose(ps, oT[:, ch * 128:(ch + 1) * 128], c["ident"])
            nc.scalar.copy(out=o_sb[:, ch, :], in_=ps)
        nc.sync.dma_start(out=out_d[:, :].rearrange("(c p) d -> p c d", p=128), in_=o_sb)


_NC = None


def _get_nc():
    global _NC
    if _NC is None:
        _NC = build_nc(T=8)
    return _NC


def host_weights(inputs):
    f = lambda k: np.asarray(inputs[k], np.float32)
    w = {
        "wqr": f("Wq_r"), "wqi": f("Wq_i"), "wqin": -f("Wq_i"),
        "wkr": f("Wk_r"), "wki": f("Wk_i"), "wkin": -f("Wk_i"),
        "wv1": np.concatenate([f("Wv_r"), f("Wv_i")], 1),
        "wv2": np.concatenate([-f("Wv_i"), f("Wv_r")], 1),
        "mr": np.float32(0.1) * f("Mv_r"), "mi": np.float32(0.1) * f("Mv_i"),
        "min": np.float32(-0.1) * f("Mv_i"),
        "wc1": f("Wc")[:D], "wc2": f("Wc")[D:],
        "bch": (np.float32(0.5) * f("bc")).reshape(3, 1),
        "wo1": f("Wo")[:D], "wo2": f("Wo")[D:],
        "bo": f("bo").reshape(D, 1),
        "ident": np.eye(D, dtype=np.float32),
        "ones_c": np.ones((128, 1), np.float32),
        "ones_r": np.ones((1, 128), np.float32),
        "ones3": np.ones((3, 1), np.float32),
    }
    return {k: np.ascontiguousarray(v, dtype=np.float32) for k, v in w.items()}


def kernel(**inputs):
    nc = _get_nc()
    w = host_weights(inputs)
    xr = np.ascontiguousarray(np.asarray(inputs["x_real"], np.float32))
    xi = np.ascontiguousarray(np.asarray(inputs["x_imag"], np.float32))
    B = xr.shape[0]
    in_maps = []
    for b in range(B):
        m = dict(w)
        m["xr"] = np.ascontiguousarray(xr[b])
        m["xi"] = np.ascontiguousarray(xi[b])
        in_maps.append(m)
    res = run_bass_kernel_spmd(nc, in_maps, core_ids=list(range(B)))
    kernel._last_results = res
    return np.stack([res.results[b]["out"] for b in range(B)], 0).astype(np.float32)


# revision 10
# speedup vs baseline: 1.0511x; 1.0511x over previous
"""Trainium2 Bass kernel for nn_ComplexRecurrentSequenceNetwork.

Self-contained: builds, compiles and runs the Bass kernel on 8 NeuronCores,
data-parallel over the batch dim (1 batch element per core, weights replicated).

Algebraic collapse used (verified vs the jax reference to fp32 noise level):
  - All K=16 stack slots stay identical (update is slot-uniform from zero init),
    so the memory attention reduces to the Mv complex-linear of a single [N,2D]
    state; Mq/Mk are dead weights.
  - The pointer only enters through its sum s, with s_{t+1} = s_t*gs/(gs+eps);
    |1-s| <= ~1e-5 over 8 steps, so s==1 (error far below fp32 noise).
  - read = 0.1*Mv(m)*s tracked directly as state V: V' = (1-p)V + p*w,
    w = 0.1*Mv(zf); z' = zf + V'.

Numerics: main path (q/k/v projections, scores, exp, AV, rowsum) in fp32
(the recurrence is chaotic: ~2.5x/step error growth; fp32r there gives ~3%
final error, fp32 gives ~7e-4).  Gate/w/broadcast matmuls in fp32r (verified
insensitive).  Softmax max-pass in bf16 (shift only needs +-40 accuracy),
with the row-max folded into the transposed-scores matmul as an appended
rank-1 contraction term.
"""
import sys

try:
    import concourse.bass as bass  # noqa: F401
except ImportError:
    sys.path.insert(0, "/opt/trn_rl_repo")

import numpy as np

import concourse.bacc as bacc
import concourse.bass as bass
import concourse.mybir as mybir
import concourse.tile as tile
from concourse.alu_op_type import AluOpType
from concourse.bass_utils import run_bass_kernel_spmd

F32 = mybir.dt.float32
F32R = mybir.dt.float32r
BF16 = mybir.dt.bfloat16
AF = mybir.ActivationFunctionType
AX = mybir.AxisListType

S, D, TWO_D, NB = 1024, 128, 256, 2   # seq len, dim, 2*dim, 512-col blocks
BLK = S // NB                          # 512
JC = S // 128                          # 8 token chunks of 128
SCALE = float(D) ** -0.5
EPS = 1e-6


def _r(ap):
    return ap.bitcast(F32R)


def build_nc(T=8):
    nc = bacc.Bacc("TRN2", target_bir_lowering=False, debug=False)

    dram = {}
    def din(name, shape):
        dram[name] = nc.dram_tensor(name, shape, F32, kind="ExternalInput")
        return dram[name]

    xr_d = din("xr", [S, D]); xi_d = din("xi", [S, D])
    # q/k stationary weights ([in, out]); *_n = negated imag part
    for n in ["wqr", "wqi", "wqin", "wkr", "wki", "wkin"]:
        din(n, [D, D])
    din("wv1", [D, TWO_D]); din("wv2", [D, TWO_D])       # v moving weights
    for n in ["mr", "mi", "min"]:                         # 0.1*Mv variants
        din(n, [D, D])
    din("wc1", [D, 3]); din("wc2", [D, 3]); din("bch", [3, 1])
    din("wo1", [D, D]); din("wo2", [D, D]); din("bo", [D, 1])
    din("ident", [D, D])
    din("ones_c", [128, 1])                               # rowsum lhsT (fp32)
    din("ones_r", [1, 128])                               # bcast/append lhsT
    din("ones3", [3, 1])
    out_d = nc.dram_tensor("out", [S, D], F32, kind="ExternalOutput")

    with tile.TileContext(nc) as tc:
        _emit(nc, tc, dram, out_d, T)
    nc.compile()
    return nc


def _t(pool, shape, dtype, tag):
    return pool.tile(shape, dtype, tag=tag, name=tag)


def _emit(nc, tc, dram, out_d, T):
    import contextlib
    ctx = contextlib.ExitStack()
    with ctx:
        cp = ctx.enter_context(tc.tile_pool(name="consts", bufs=1))
        st = ctx.enter_context(tc.tile_pool(name="state", bufs=1))
        wk = ctx.enter_context(tc.tile_pool(name="work", bufs=1))
        rowp = ctx.enter_context(tc.tile_pool(name="rows", bufs=2))
        pmm = ctx.enter_context(tc.tile_pool(name="pmm", bufs=2, space="PSUM"))
        pacc = ctx.enter_context(tc.tile_pool(name="pacc", bufs=4, space="PSUM"))
        paux = ctx.enter_context(tc.tile_pool(name="paux", bufs=2, space="PSUM"))

        # ---- load constants ----
        c = {}
        for n, sh in [("wqr", [D, D]), ("wqi", [D, D]), ("wqin", [D, D]),
                      ("wkr", [D, D]), ("wki", [D, D]), ("wkin", [D, D]),
                      ("wv1", [D, TWO_D]), ("wv2", [D, TWO_D]),
                      ("wo1", [D, D]), ("wo2", [D, D]),
                      ("ident", [D, D]),
                      ("ones_r", [1, 128]),
                      ("bch", [3, 1]), ("bo", [D, 1])]:
            c[n] = _t(cp, sh, F32, n)
            nc.sync.dma_start(out=c[n], in_=dram[n][:, :])
        # f32r-rounded constants (producers must round for fp32r matmuls)
        for n, sh in [("mr", [D, D]), ("mi", [D, D]), ("min", [D, D]),
                      ("wc1", [D, 3]), ("wc2", [D, 3]), ("ones3", [3, 1]),
                      ("ones_c", [128, 1])]:
            raw = _t(cp, sh, F32, n + "_raw")
            nc.sync.dma_start(out=raw, in_=dram[n][:, :])
            c[n] = _t(cp, sh, F32, n)
            nc.vector.tensor_copy(out=_r(c[n]), in_=raw)
        c["ones_rb"] = _t(cp, [1, 128], BF16, "ones_rb")
        nc.vector.tensor_copy(out=c["ones_rb"], in_=c["ones_r"])

        # ---- state (parity buffered) ----
        z = [[_t(st, [128, S], F32, f"z{p}{comp}") for comp in "ri"] for p in (0, 1)]
        V = [[_t(st, [128, S], F32, f"V{p}{comp}") for comp in "ri"] for p in (0, 1)]
        for comp in (0, 1):
            nc.vector.memset(V[0][comp], 0.0)

        # ---- load + transpose x into z[0] ([feat, tok] layout) ----
        for comp, src in [(0, dram["xr"]), (1, dram["xi"])]:
            x_sb = _t(wk, [128, JC, D], F32, "xin")
            nc.sync.dma_start(out=x_sb, in_=src.rearrange("(c p) d -> p c d", p=128))
            for ch in range(JC):
                ps = _t(pmm, [128, 128], F32, "mm")
                nc.tensor.transpose(ps, x_sb[:, ch, :], c["ident"])
                nc.scalar.copy(out=z[0][comp][:, ch * 128:(ch + 1) * 128], in_=ps)

        # ---- work tiles ----
        qks = _t(wk, [128, S], F32, "qks")   # shared fp32 scratch (hi+lo is exact)
        qk_hi = {n: _t(wk, [128, S], F32, n + "h") for n in ["qrT", "qiT", "krT", "kiT"]}
        qk_lo = {n: _t(wk, [128, S], F32, n + "l") for n in ["qrT", "qiT", "krT", "kiT"]}
        qk_bf = {n: _t(wk, [128, S], BF16, n + "bf") for n in ["qrT", "qiT", "krT", "kiT"]}
        v_sb = _t(wk, [128, JC, TWO_D], F32, "vsb")
        e_sb = _t(wk, [128, JC, S], F32, "esb")
        mcol = _t(wk, [128, JC], F32, "mcol")
        zf = [_t(wk, [128, S], F32, f"zf{comp}") for comp in "ri"]
        zfr_ = [_t(wk, [128, S], F32, f"zfr{comp}") for comp in "ri"]  # f32r copies
        tg = _t(wk, [3, S], F32, "tg")
        negm = _t(wk, [1, S], BF16, "negm")
        dtl = [_t(wk, [128, S], F32, f"d{comp}") for comp in "ri"]
        t2l = [_t(wk, [128, S], F32, f"t2{comp}") for comp in "ri"]
        rcpb = _t(wk, [128, S], F32, "rcpb")

        for t in range(T):
            za, Va = z[t % 2], V[t % 2]
            zb, Vb = z[(t + 1) % 2], V[(t + 1) % 2]

            # --- phase A: q/k projections (fp32) + hi/lo split per dst ---
            specs = [("qrT", "wqr", 0, "wqin", 1), ("qiT", "wqi", 0, "wqr", 1),
                     ("krT", "wkr", 0, "wkin", 1), ("kiT", "wki", 0, "wkr", 1)]
            for dst, wA, sA, wB, sB in specs:
                for b in range(NB):
                    sl = slice(b * BLK, (b + 1) * BLK)
                    ps = _t(pmm, [128, BLK], F32, "mm")
                    nc.tensor.matmul(ps, c[wA], za[sA][:, sl], start=True, stop=False)
                    nc.tensor.matmul(ps, c[wB], za[sB][:, sl], start=False, stop=True)
                    nc.scalar.copy(out=qks[:, sl], in_=ps)
                # hi/lo: scores become 3 f32r terms (26-bit effective, 1cyc/row)
                nc.vector.tensor_copy(out=_r(qk_hi[dst]), in_=qks)
                nc.vector.tensor_tensor(out=_r(qk_lo[dst]), in0=qks, in1=qk_hi[dst],
                                        op=AluOpType.subtract)
                nc.gpsimd.tensor_copy(out=qk_bf[dst], in_=qk_hi[dst])

            # --- phase B: v natural layout [tok, 2D] (fp32) ---
            for ch in range(JC):
                csl = slice(ch * 128, (ch + 1) * 128)
                ps = _t(pmm, [128, TWO_D], F32, "mm")
                nc.tensor.matmul(ps, za[0][:, csl], c["wv1"], start=True, stop=False)
                nc.tensor.matmul(ps, za[1][:, csl], c["wv2"], start=False, stop=True)
                nc.scalar.copy(out=v_sb[:, ch, :], in_=ps)

            # --- phase C: bf16 max-pass, scores in [i, j] layout ---
            for ic in range(JC):
                isl = slice(ic * 128, (ic + 1) * 128)
                r01 = _t(rowp, [128, 2], F32, "mr2")
                for jb in range(NB):
                    jsl = slice(jb * BLK, (jb + 1) * BLK)
                    ps = _t(pmm, [128, BLK], F32, "mm")
                    nc.tensor.matmul(ps, qk_bf["qrT"][:, isl], qk_bf["krT"][:, jsl],
                                     start=True, stop=False)
                    nc.tensor.matmul(ps, qk_bf["qiT"][:, isl], qk_bf["kiT"][:, jsl],
                                     start=False, stop=True)
                    nc.vector.tensor_reduce(out=r01[:, jb:jb + 1], in_=ps,
                                            axis=AX.X, op=AluOpType.max)
                nc.vector.tensor_tensor(out=mcol[:, ic:ic + 1], in0=r01[:, 0:1],
                                        in1=r01[:, 1:2], op=AluOpType.max)
            # row-max columns -> one row [1, S] (PE transpose), negated f32r copy
            for b in range(NB):
                ps_row = _t(paux, [1, BLK], F32, "aux")
                for k in range(4):
                    ic = b * 4 + k
                    nc.tensor.transpose(ps_row[0:1, k * 128:(k + 1) * 128],
                                        mcol[:, ic:ic + 1], c["ident"])
                nc.scalar.activation(out=negm[0:1, b * BLK:(b + 1) * BLK],
                                     in_=ps_row, func=AF.Copy, scale=-1.0)

            # --- phase D: transposed scores + shift + exp (fp32 + f32r shift) ---
            for ch in range(JC):
                csl = slice(ch * 128, (ch + 1) * 128)
                for b in range(NB):
                    sl = slice(b * BLK, (b + 1) * BLK)
                    ps = _t(pmm, [128, BLK], F32, "mm")
                    first = True
                    for kk, qq in [("krT", "qrT"), ("kiT", "qiT")]:
                        kh, kl = _r(qk_hi[kk][:, csl]), _r(qk_lo[kk][:, csl])
                        qh, ql = _r(qk_hi[qq][:, sl]), _r(qk_lo[qq][:, sl])
                        nc.tensor.matmul(ps, kh, qh, start=first, stop=False)
                        nc.tensor.matmul(ps, kh, ql, start=False, stop=False)
                        nc.tensor.matmul(ps, kl, qh, start=False, stop=False)
                        first = False
                    nc.tensor.matmul(ps, c["ones_rb"], negm[0:1, sl],
                                     start=False, stop=True)
                    nc.scalar.activation(out=_r(e_sb[:, ch, sl]), in_=ps,
                                         func=AF.Exp, scale=SCALE)

            # --- phase E: AV + rowsum (fp32), accumulate over j chunks ---
            ar_ps, rs_ps = [], []
            for b in range(NB):
                sl = slice(b * BLK, (b + 1) * BLK)
                pr = _t(pacc, [128, BLK], F32, "acc")
                pi = _t(pacc, [128, BLK], F32, "acc")
                rs = _t(paux, [1, BLK], F32, "aux")
                for ch in range(JC):
                    e = e_sb[:, ch, sl]
                    nc.tensor.matmul(pr, v_sb[:, ch, 0:D], e,
                                     start=(ch == 0), stop=(ch == JC - 1))
                    nc.tensor.matmul(pi, v_sb[:, ch, D:TWO_D], e,
                                     start=(ch == 0), stop=(ch == JC - 1))
                    nc.tensor.matmul(rs, _r(c["ones_c"]), _r(e),
                                     start=(ch == 0), stop=(ch == JC - 1))
                ar_ps.append((pr, pi)); rs_ps.append(rs)

            # --- phase F: normalize -> zf (+ f32r copies for gate/w matmuls) ---
            for b in range(NB):
                sl = slice(b * BLK, (b + 1) * BLK)
                rcp = _t(rowp, [1, BLK], F32, "rcp")
                nc.vector.reciprocal_approx_fast(out=rcp, in_=rs_ps[b])
                psb = _t(paux, [128, BLK], F32, "aux")
                nc.tensor.matmul(psb, c["ones_r"], rcp, start=True, stop=True)
                nc.scalar.copy(out=rcpb[:, sl], in_=psb)
                for comp in (0, 1):
                    nc.vector.tensor_tensor(out=zf[comp][:, sl], in0=ar_ps[b][comp],
                                            in1=rcpb[:, sl], op=AluOpType.mult)
                    nc.vector.tensor_copy(out=_r(zfr_[comp][:, sl]), in_=zf[comp][:, sl])

            # --- phase G: gates (f32r) ---
            for b in range(NB):
                sl = slice(b * BLK, (b + 1) * BLK)
                gp = _t(pmm, [3, BLK], F32, "mm")
                nc.tensor.matmul(gp, _r(c["wc1"]), _r(zfr_[0][:, sl]), start=True, stop=False)
                nc.tensor.matmul(gp, _r(c["wc2"]), _r(zfr_[1][:, sl]), start=False, stop=True)
                nc.scalar.activation(out=_r(tg[:, sl]), in_=gp, func=AF.Tanh,
                                     scale=0.5, bias=c["bch"])
                tp = _t(paux, [1, BLK], F32, "aux")
                nc.tensor.matmul(tp, _r(c["ones3"]), _r(tg[:, sl]), start=True, stop=True)
                drow = _t(rowp, [1, BLK], F32, "drow")
                nc.vector.tensor_scalar_add(out=drow, in0=tp, scalar1=float(3.0 + 2 * EPS))
                rd = _t(rowp, [1, BLK], F32, "rd")
                nc.vector.reciprocal_approx_fast(out=rd, in_=drow)
                prow = _t(rowp, [1, BLK], F32, "prow")
                nc.vector.scalar_tensor_tensor(out=prow, in0=tg[0:1, sl], scalar=1.0,
                                               in1=rd, op0=AluOpType.add, op1=AluOpType.mult)
                pb = _t(paux, [128, BLK], F32, "aux")
                nc.tensor.matmul(pb, c["ones_r"], prow, start=True, stop=True)

                # --- phase H: w (f32r) + V update + z' ---
                wps = []
                for wA, wB in [("mr", "min"), ("mi", "mr")]:
                    wp = _t(pmm, [128, BLK], F32, "mm")
                    nc.tensor.matmul(wp, _r(c[wA]), _r(zfr_[0][:, sl]), start=True, stop=False)
                    nc.tensor.matmul(wp, _r(c[wB]), _r(zfr_[1][:, sl]), start=False, stop=True)
                    wps.append(wp)
                for comp in (0, 1):
                    nc.vector.tensor_tensor(out=dtl[comp][:, sl], in0=wps[comp],
                                            in1=Va[comp][:, sl], op=AluOpType.subtract)
                    nc.vector.tensor_tensor(out=t2l[comp][:, sl], in0=dtl[comp][:, sl],
                                            in1=pb, op=AluOpType.mult)
                    nc.gpsimd.tensor_tensor(out=Vb[comp][:, sl], in0=Va[comp][:, sl],
                                            in1=t2l[comp][:, sl], op=AluOpType.add)
                    nc.vector.tensor_tensor(out=zb[comp][:, sl], in0=zf[comp][:, sl],
                                            in1=Vb[comp][:, sl], op=AluOpType.add)

        # ---- epilogue: out = [zr|zi] @ Wo + bo, back to [tok, feat] ----
        zfin = z[T % 2]
        oT = _t(wk, [128, S], F32, "oT")
        for b in range(NB):
            sl = slice(b * BLK, (b + 1) * BLK)
            ps = _t(pmm, [128, BLK], F32, "mm")
            nc.tensor.matmul(ps, c["wo1"], zfin[0][:, sl], start=True, stop=False)
            nc.tensor.matmul(ps, c["wo2"], zfin[1][:, sl], start=False, stop=True)
            nc.scalar.activation(out=oT[:, sl], in_=ps, func=AF.Identity, bias=c["bo"])
        o_sb = _t(wk, [128, JC, D], F32, "onat")
        for ch in range(JC):
            ps = _t(pmm, [128, 128], F32, "mm")
            nc.tensor.transpose(ps, oT[:, ch * 128:(ch + 1) * 128], c["ident"])
            nc.scalar.copy(out=o_sb[:, ch, :], in_=ps)
        nc.sync.dma_start(out=out_d[:, :].rearrange("(c p) d -> p c d", p=128), in_=o_sb)


_NC = None


def _get_nc():
    global _NC
    if _NC is None:
        _NC = build_nc(T=8)
    return _NC


def host_weights(inputs):
    f = lambda k: np.asarray(inputs[k], np.float32)
    w = {
        "wqr": f("Wq_r"), "wqi": f("Wq_i"), "wqin": -f("Wq_i"),
        "wkr": f("Wk_r"), "wki": f("Wk_i"), "wkin": -f("Wk_i"),
        "wv1": np.concatenate([f("Wv_r"), f("Wv_i")], 1),
        "wv2": np.concatenate([-f("Wv_i"), f("Wv_r")], 1),
        "mr": np.float32(0.1) * f("Mv_r"), "mi": np.float32(0.1) * f("Mv_i"),
        "min": np.float32(-0.1) * f("Mv_i"),
        "wc1": f("Wc")[:D], "wc2": f("Wc")[D:],
        "bch": (np.float32(0.5) * f("bc")).reshape(3, 1),
        "wo1": f("Wo")[:D], "wo2": f("Wo")[D:],
        "bo": f("bo").reshape(D, 1),
        "ident": np.eye(D, dtype=np.float32),
        "ones_c": np.ones((128, 1), np.float32),
        "ones_r": np.ones((1, 128), np.float32),
        "ones3": np.ones((3, 1), np.float32),
    }
    return {k: np.ascontiguousarray(v, dtype=np.float32) for k, v in w.items()}


def kernel(**inputs):
    nc = _get_nc()
    w = host_weights(inputs)
    xr = np.ascontiguousarray(np.asarray(inputs["x_real"], np.float32))
    xi = np.ascontiguousarray(np.asarray(inputs["x_imag"], np.float32))
    B = xr.shape[0]
    in_maps = []
    for b in range(B):
        m = dict(w)
        m["xr"] = np.ascontiguousarray(xr[b])
        m["xi"] = np.ascontiguousarray(xi[b])
        in_maps.append(m)
    res = run_bass_kernel_spmd(nc, in_maps, core_ids=list(range(B)))
    kernel._last_results = res
    return np.stack([res.results[b]["out"] for b in range(B)], 0).astype(np.float32)
